# revision 64
# baseline (speedup 1.0000x reference)
"""MHA kernel for TRN2: x[8,512,32,32], 8 heads, S=1024, C=512.

Sharding: data-parallel over batch N=8 -> one batch item per NeuronCore.
Per-core layout (all transpose-free):
  qkT[e,s]  = w_qkvT[:, :1024].T @ x      (e on partitions; q tiles 0-3, k tiles 4-7)
  v[s,e]    = x.T @ w_qkvT[:, 1024:]      (s on partitions, natural layout)
  scoresT   = kT_h.T @ qT_h               (k_s on partitions; K=64 -> head pair packed
                                           at array rows 0-63 / 64-127, runs 2x via
                                           implicit 64x128 row tiling)
  P         = exp(scoresT * 1/8)          (ACT, batched 2048-wide from PSUM)
  oT_aug    = [v_h | 1].T @ P             (M=65; row 64 = softmax denominator r)
  oT        = oT_aug[:64] * (1/r)         (gpsimd partition_broadcast of 1/r)
  yT[o,s]   = w_outT.T @ oT               (+ b_out added host-side; == NCHW layout)

Perf structure (116.0us baseline -> ~111.9us):
  - inputs are host-preswizzled into contiguous-per-partition layouts and land
    in 4 consolidated SBUF tiles via 2KB-element DMA slices in needed-first
    order (wqa -> xa -> wqb/wv/wo on sync; xb on the scalar HWDGE queue):
    sub-2KB descriptors or >2KB elements both collapse DMA queue throughput
  - 12 warm-up matmuls on a zero tile bridge the DMA window so the PE HAM
    clock-gate is already 8/8 when the real stream starts
  - mid-kernel normalization: one staging copy [65,512] frees the pso bank in
    ~0.7us (the next step's PV accumulators are gated on it); all four staging
    copies are emitted before any finish chain so the in-order DVE releases
    banks back-to-back. The r row is re-staged to partition 0 of its own tile
    because custom-DVE ops and partition_broadcast ignore AP partition offsets
  - final drain: pair-3 PV runs the nt=1 indices first (bank-alternating; a
    per-bank sweep serializes on same-bank accumulation), norm chains read
    PSUM directly, st1 reuses the drained pso_t tiles in place; st0 ct0..2
    and the st1 groups fill the PE after the PV sweeps, and st0's ct=3 closes
    as two 64x128 row-tiled halves gated on one chain each. Final y copies
    split across ACT and DVE, all y DMAs on the idle sync queue
"""

import numpy as np
import ml_dtypes

import concourse.bacc as bacc
import concourse.mybir as mybir
import concourse.tile as tile
from concourse.bass_utils import run_bass_kernel_spmd

P = 128
S = 1024          # sequence = 32*32
C = 512           # channels
NH = 8            # heads
HD = 64           # head dim
CT = C // P       # 4 c-tiles
ET = 2 * C // P   # 8 e-tiles for q+k
MT = S // P       # 8 s-tiles
BF = mybir.dt.bfloat16
F32 = mybir.dt.float32

_cache = {}


def build_program(dbg=False):
    nc = bacc.Bacc("TRN2", target_bir_lowering=False, debug=False, num_devices=8)
    # inputs come pre-swizzled from the host so every DMA is a contiguous
    # >=2KB-per-partition read (small descriptors collapse queue throughput):
    #   x:  [p, nt, ct, 512]  = x[ct*128+p, nt*512+s]
    #   wq: [p, eo, ct, 128]  = w_qkvT[ct*128+p, ET_ORDER[eo]*128+e]
    #   wv: [p, ct, 512]      = w_qkvT[ct*128+p, 1024+e]
    #   wo: [p, ct, 512]      = w_outT[ct*128+p, o]
    xa_d = nc.dram_tensor("xa", [P, CT * 512], BF, kind="ExternalInput").ap()
    xb_d = nc.dram_tensor("xb", [P, CT * 512], BF, kind="ExternalInput").ap()
    wqa_d = nc.dram_tensor("wqa", [P, 2 * CT * P], BF, kind="ExternalInput").ap()
    wqb_d = nc.dram_tensor("wqb", [P, 6 * CT * P], BF, kind="ExternalInput").ap()
    wv_d = nc.dram_tensor("wv", [P, CT * C], BF, kind="ExternalInput").ap()
    wo_d = nc.dram_tensor("wo", [P, CT * C], BF, kind="ExternalInput").ap()
    y_d = nc.dram_tensor("y", [C, S], BF, kind="ExternalOutput").ap()

    with tile.TileContext(nc) as tc:
        with (
            tc.tile_pool(name="const", bufs=1) as cpool,
            tc.tile_pool(name="qk", bufs=1) as qkpool,
            tc.tile_pool(name="vp", bufs=1) as vpool,
            tc.tile_pool(name="pp", bufs=32) as ppool,
            tc.tile_pool(name="ot", bufs=1) as opool,
            tc.tile_pool(name="yp", bufs=1) as ypool,
            tc.tile_pool(name="misc", bufs=4) as mpool,
            tc.tile_pool(name="psq", bufs=2, space="PSUM") as psq_pool,
            tc.tile_pool(name="pso", bufs=4, space="PSUM") as pso_pool,
        ):
            # ---- warm-up: zero tile + dummy matmuls keep the PE busy while
            # input DMAs stream, so HAM is un-throttled for the real work ----
            wz = cpool.tile([P, 512], BF, name="wz", tag="wz")
            nc.gpsimd.memset(wz[:], 0.0)
            wu = psq_pool.tile([P, 1024], F32, name="wu", tag="psq")
            for _ in range(12):
                nc.tensor.matmul(wu[:, 0:512], wz[:, 0:128], wz[:], start=True, stop=True)

            # ---- consolidated input tiles, same layout as the prepped DRAM ----
            w4 = cpool.tile([P, ET * CT * P], BF, name="w4", tag="w4")
            x4 = cpool.tile([P, 2 * CT * 512], BF, name="x4", tag="x4")
            wv4 = cpool.tile([P, CT * C], BF, name="wv4", tag="wv4")
            wo4 = cpool.tile([P, CT * C], BF, name="wo4", tag="wo4")

            # One queue, strict priority order; every dma_start moves a
            # 1024-column (2KB-per-partition) slice - that elem size keeps the
            # queue streaming back-to-back, and a second parallel queue only
            # splits HBM bandwidth without improving the critical chain.
            nc.sync.dma_start(w4[:, 0:1024], wqa_d[:])             # et 0 and 4
            nc.sync.dma_start(x4[:, 0:1024], xa_d[:, 0:1024])      # s 0:512
            nc.sync.dma_start(x4[:, 1024:2048], xa_d[:, 1024:2048])
            for i in range(2):
                sl = slice(i * 1024, (i + 1) * 1024)
                nc.scalar.dma_start(x4[:, 2048 + i * 1024:3072 + i * 1024],
                                    xb_d[:, sl])                   # s 512:1024
            for i in range(3):                                     # et 1,5|2,6|3,7
                sl = slice(i * 1024, (i + 1) * 1024)
                nc.sync.dma_start(w4[:, 1024 + i * 1024:2048 + i * 1024], wqb_d[:, sl])
            for i in range(2):
                sl = slice(i * 1024, (i + 1) * 1024)
                nc.sync.dma_start(wv4[:, sl], wv_d[:, sl])
            for i in range(2):
                sl = slice(i * 1024, (i + 1) * 1024)
                nc.sync.dma_start(wo4[:, sl], wo_d[:, sl])

            ET_ORDER = (0, 4, 1, 5, 2, 6, 3, 7)
            ET_OFF = {et: i for i, et in enumerate(ET_ORDER)}

            def w_slice(et, ct):
                o = (ET_OFF[et] * CT + ct) * P
                return w4[:, o:o + P]

            def x_nt(ct, nt):
                o = (nt * CT + ct) * 512
                return x4[:, o:o + 512]

            def x_mt(ct, mt):
                nt, r = divmod(mt, 4)
                o = (nt * CT + ct) * 512 + r * P
                return x4[:, o:o + P]

            def wv_sb(ct):
                return wv4[:, ct * C:(ct + 1) * C]

            def wo_sb(ct):
                return wo4[:, ct * C:(ct + 1) * C]

            # ---- qkT projection: [e=1024 rows, s=1024] ----
            qk_sb = []
            for et in range(ET):
                t = qkpool.tile([P, S], BF, name=f"qk{et}", tag=f"qk{et}")
                qk_sb.append(t)
            v_sb = [None] * MT

            def emit_qkv_group(et, nt):
                ps = pso_pool.tile([P, 512], F32, name="qp", tag="pso")
                for ct in range(CT):
                    nc.tensor.matmul(
                        ps[:],
                        w_slice(et, ct),
                        x_nt(ct, nt),
                        start=(ct == 0), stop=(ct == CT - 1),
                    )
                nc.vector.tensor_copy(qk_sb[et][:, nt * 512:(nt + 1) * 512], ps[:])

            def emit_v_group(mt):
                ps = pso_pool.tile([P, 512], F32, name="vp", tag="pso")
                for ct in range(CT):
                    nc.tensor.matmul(
                        ps[:],
                        x_mt(ct, mt),
                        wv_sb(ct),
                        start=(ct == 0), stop=(ct == CT - 1),
                    )
                vt = vpool.tile([P, NH * (HD + 1)], BF, name=f"v{mt}", tag=f"v{mt}")
                vv = vt[:].rearrange("p (h e) -> p h e", e=HD + 1)
                nc.gpsimd.memset(vv[:, :, HD:HD + 1], 1.0)
                nc.vector.tensor_copy(
                    vv[:, :, 0:HD], ps[:].rearrange("p (h e) -> p h e", e=HD))
                v_sb[mt] = vt

            # block A: only the nt0 tiles gate the first QK/exp; the nt1
            # groups are emitted right after the first ACT (see step loop)
            for et, nt in ((0, 0), (4, 0)):
                emit_qkv_group(et, nt)
            pending = [("qkv", et, nt) for et in (1, 5, 2, 6, 3, 7) for nt in (0, 1)]
            pending += [("v", mt, None) for mt in range(MT)]
            pend_i = 0

            # ---- attention, software-pipelined: QK/exp(pair p) || PV(pair p-1);
            #      step 0 also drains the remaining qkv/v projection groups ----
            oT_sb = [opool.tile([P, S], BF, name=f"o{ct}", tag=f"o{ct}") for ct in range(CT)]
            p_tiles = {}
            DRAIN_ORDER = ((0, 0), (1, 0), (0, 1), (1, 1))
            LAST_ORDER = ((0, 1), (1, 1), (0, 0), (1, 0))

            def norm_stage(pso_t, idx):
                # phase A: one fast DVE copy stages the whole PSUM tile to
                # SBUF so the pso bank frees in ~0.7us (the next step's PV
                # accumulations are gated on it)
                stage = mpool.tile([HD + 1, 512], F32, name="stage", tag="stage")
                nc.vector.tensor_copy(stage[:], pso_t[idx][0:HD + 1, :])
                return stage

            def norm_finish_pre(stage):
                # phase B1 off SBUF: the r row is re-staged to partition 0 of
                # its own tile (custom-DVE ops and partition_broadcast read
                # partition 0 of the underlying tile regardless of slicing)
                rrow = mpool.tile([1, 512], F32, name="rrow", tag="rrow")
                nc.vector.tensor_copy(rrow[0:1, :], stage[HD:HD + 1, :])
                rinv = mpool.tile([1, 512], F32, name="rinv", tag="rinv")
                nc.vector.reciprocal_approx_fast(rinv[0:1, :], rrow[0:1, :])
                bc = mpool.tile([HD, 512], F32, name="bc", tag="bc")
                nc.gpsimd.partition_broadcast(bc[:], rinv[0:1, :], channels=HD)
                return bc

            def norm_finish_mul(step, idx_order_idx, stage, bc, order):
                pp = step - 1
                hh, nt = order[idx_order_idx]
                h = 2 * pp + hh
                ct, half = h // 2, h % 2
                nc.vector.tensor_mul(
                    oT_sb[ct][half * HD:(half + 1) * HD, nt * 512:(nt + 1) * 512],
                    stage[0:HD, :], bc[:],
                )

            def norm_finish(step, idx_order_idx, stage, order):
                bc = norm_finish_pre(stage)
                norm_finish_mul(step, idx_order_idx, stage, bc, order)

            def emit_norm(step, idx_order_idx, pso_t, order):
                stage = norm_stage(pso_t, idx_order_idx)
                norm_finish(step, idx_order_idx, stage, order)

            def norm_direct_pre(pso_t, idx):
                # drain-tail variant: no staging (nothing else wants the pso
                # bank sooner than the chain finishes) - rrow/recip only
                rrow = mpool.tile([1, 512], F32, name="rrow", tag="rrow")
                nc.vector.tensor_copy(rrow[0:1, :], pso_t[idx][HD:HD + 1, :])
                rinv = mpool.tile([1, 512], F32, name="rinv", tag="rinv")
                nc.vector.reciprocal_approx_fast(rinv[0:1, :], rrow[0:1, :])
                bc = mpool.tile([HD, 512], F32, name="bc", tag="bc")
                nc.gpsimd.partition_broadcast(bc[:], rinv[0:1, :], channels=HD)
                return bc

            def norm_direct_mul(step, idx_order_idx, pso_t, bc, order):
                pp = step - 1
                hh, nt = order[idx_order_idx]
                h = 2 * pp + hh
                ct, half = h // 2, h % 2
                nc.vector.tensor_mul(
                    oT_sb[ct][half * HD:(half + 1) * HD, nt * 512:(nt + 1) * 512],
                    pso_t[idx_order_idx][0:HD, :], bc[:],
                )

            y_sb = [ypool.tile([P, S], BF, name=f"y{ot}", tag=f"y{ot}") for ot in range(CT)]

            for step in range(NH // 2):
                pso_t = None
                if step >= 1:
                    pso_t = [pso_pool.tile([P, 512], F32, name=f"pso{i}", tag="pso")
                             for i in range(4)]
                for mt in range(MT):
                    for nt in range(2):
                        psq = psq_pool.tile([P, 1024], F32, name="psq", tag="psq")
                        for hh in range(2):
                            nc.tensor.matmul(
                                psq[:, hh * 512:(hh + 1) * 512],
                                qk_sb[4 + step][hh * HD:(hh + 1) * HD, mt * P:(mt + 1) * P],
                                qk_sb[step][hh * HD:(hh + 1) * HD, nt * 512:(nt + 1) * 512],
                                start=True, stop=True,
                            )
                        pt = ppool.tile([P, 1024], BF, name="ptile", tag="ptile")
                        nc.scalar.activation(
                            pt[:], psq[:], mybir.ActivationFunctionType.Exp,
                            scale=float(1.0 / np.sqrt(HD)),
                        )
                        p_tiles[(step, mt, nt)] = pt
                        if step == 0 and mt == 0 and nt == 0:
                            emit_qkv_group(0, 1)
                            emit_qkv_group(4, 1)
                        if step == 0:
                            slot = mt * 2 + nt
                            want = 20 * (slot + 1) // 16
                            while pend_i < min(want, 20):
                                kind, i1, i2 = pending[pend_i]
                                if kind == "qkv":
                                    emit_qkv_group(i1, i2)
                                else:
                                    emit_v_group(i1)
                                pend_i += 1
                    if step >= 1:
                        pp = step - 1
                        for idx, (hh, nt) in enumerate(DRAIN_ORDER):
                            h = 2 * pp + hh
                            nc.tensor.matmul(
                                pso_t[idx][0:HD + 1, :],
                                v_sb[mt][:, h * (HD + 1):(h + 1) * (HD + 1)],
                                p_tiles[(pp, mt, nt)][:, hh * 512:(hh + 1) * 512],
                                start=(mt == 0), stop=(mt == MT - 1),
                            )
                if step >= 1:
                    # all four staging copies first: they are what frees the
                    # pso ring for the next step's PV; the serial finish
                    # chains would otherwise delay stages 2/3 on the DVE FIFO
                    stages = [norm_stage(pso_t, i) for i in range(4)]
                    for i in range(4):
                        norm_finish(step, i, stages[i], DRAIN_ORDER)

            # ---- pair-3 drain + output projection ----
            # PV runs in two passes over mt: the nt=1 drain indices first, so
            # their normalization chains and the st1 projection groups hide
            # under the nt=0 PV pass; st0 (columns 0:512) accumulates ct 0..2
            # on the freed psq ring meanwhile.
            pp = NH // 2 - 1
            pso_t = [pso_pool.tile([P, 512], F32, name=f"psoF{i}", tag="pso")
                     for i in range(4)]

            def pv_last(idx):
                hh, nt = LAST_ORDER[idx]
                h = 2 * pp + hh
                for mt in range(MT):
                    nc.tensor.matmul(
                        pso_t[idx][0:HD + 1, :],
                        v_sb[mt][:, h * (HD + 1):(h + 1) * (HD + 1)],
                        p_tiles[(pp, mt, nt)][:, hh * 512:(hh + 1) * 512],
                        start=(mt == 0), stop=(mt == MT - 1),
                    )

            psA = psq_pool.tile([P, 1024], F32, name="prA", tag="psq")
            psB = psq_pool.tile([P, 1024], F32, name="prB", tag="psq")

            # filler matmuls occupy the PE while step-3's staging copies
            # release the pso ring for pass A - without them the ~1.7us idle
            # window re-throttles the HAM clock-gate to 4/8 for the whole
            # drain. st0's first accumulation (start=True) overwrites psA.
            for _ in range(6):
                nc.tensor.matmul(psA[:, 0:512], wz[:, 0:128], wz[:],
                                 start=True, stop=True)

            def st0_ps(g):
                t = psA if g < 2 else psB
                return t[:, (g % 2) * 512:(g % 2 + 1) * 512]

            def emit_st1(g, ps):
                # accumulates in the drained pso_t[g] tile, in place
                for ct in range(CT):
                    nc.tensor.matmul(
                        ps[:],
                        wo_sb(ct)[:, g * P:(g + 1) * P],
                        oT_sb[ct][:, 512:1024],
                        start=(ct == 0), stop=(ct == CT - 1),
                    )
                dst = y_sb[g][:, 512:1024]
                nc.scalar.copy(dst, ps[:])
                nc.sync.dma_start(y_d[g * P:(g + 1) * P, 512:1024], dst)

            # pass A: nt=1 indices together (they share P tiles per mt)
            for mt in range(MT):
                nc.tensor.matmul(
                    pso_t[0][0:HD + 1, :],
                    v_sb[mt][:, (2 * pp) * (HD + 1):(2 * pp + 1) * (HD + 1)],
                    p_tiles[(pp, mt, 1)][:, 0:512],
                    start=(mt == 0), stop=(mt == MT - 1),
                )
                nc.tensor.matmul(
                    pso_t[1][0:HD + 1, :],
                    v_sb[mt][:, (2 * pp + 1) * (HD + 1):(2 * pp + 2) * (HD + 1)],
                    p_tiles[(pp, mt, 1)][:, 512:1024],
                    start=(mt == 0), stop=(mt == MT - 1),
                )
            # direct (no-staging) norm chains, rrow/recip pairs interleaved so
            # the DVE never waits on a gpsimd broadcast
            bc0 = norm_direct_pre(pso_t, 0)
            bc1 = norm_direct_pre(pso_t, 1)
            norm_direct_mul(NH // 2, 0, pso_t, bc0, LAST_ORDER)
            norm_direct_mul(NH // 2, 1, pso_t, bc1, LAST_ORDER)

            # pass B PV sweeps run back-to-back; every normalization chain
            # and the st0/st1 filler matmuls hide underneath them
            pv_last(2)
            bc2 = norm_direct_pre(pso_t, 2)
            norm_direct_mul(NH // 2, 2, pso_t, bc2, LAST_ORDER)
            pv_last(3)

            # st0 accumulation over ct=0..2 has no dependency on the last
            # pair; by now the step-3 norms it reads are long done
            for ct in range(CT - 1):
                for g in range(CT):
                    nc.tensor.matmul(
                        st0_ps(g),
                        wo_sb(ct)[:, g * P:(g + 1) * P],
                        oT_sb[ct][:, 0:512],
                        start=(ct == 0), stop=False,
                    )
            emit_st1(0, pso_t[0])
            emit_st1(1, pso_t[1])
            bc3 = norm_direct_pre(pso_t, 3)
            norm_direct_mul(NH // 2, 3, pso_t, bc3, LAST_ORDER)
            emit_st1(2, pso_t[2])
            emit_st1(3, pso_t[3])
            # st0's ct=3 contraction splits into row halves (64x128 row-tiled
            # matmuls) so the first half runs as soon as the idx2 chain's
            # multiply lands, instead of waiting for idx3's as well
            ct = CT - 1
            for g in range(CT):
                nc.tensor.matmul(
                    st0_ps(g),
                    wo_sb(ct)[0:HD, g * P:(g + 1) * P],
                    oT_sb[ct][0:HD, 0:512],
                    start=False, stop=False,
                )
            for g in range(CT):
                nc.tensor.matmul(
                    st0_ps(g),
                    wo_sb(ct)[HD:P, g * P:(g + 1) * P],
                    oT_sb[ct][HD:P, 0:512],
                    start=False, stop=True,
                )
            # split the final evacuation across ACT and DVE so the last four
            # copies run pairwise-parallel instead of serializing on one engine
            for g in range(CT):
                dst = y_sb[g][:, 0:512]
                if g < 2:
                    nc.scalar.copy(dst, st0_ps(g))
                else:
                    nc.vector.tensor_copy(dst, st0_ps(g))
                nc.sync.dma_start(y_d[g * P:(g + 1) * P, 0:512], dst)

    nc.compile()
    return nc


def get_program():
    if "nc" not in _cache:
        _cache["nc"] = build_program()
    return _cache["nc"]


def kernel(x, w_qkv, w_out, b_out, _trace=False, _tmpdir=None):
    x = np.asarray(x, dtype=np.float32)
    w_qkv = np.asarray(w_qkv, dtype=np.float32)
    w_out = np.asarray(w_out, dtype=np.float32)
    b_out = np.asarray(b_out, dtype=np.float32)
    N = x.shape[0]

    xb = x.reshape(N, C, S).astype(ml_dtypes.bfloat16)
    wqT = np.ascontiguousarray(w_qkv.T).astype(ml_dtypes.bfloat16)
    woT = np.ascontiguousarray(w_out.T).astype(ml_dtypes.bfloat16)

    # pre-swizzle into the contiguous-per-partition layouts the kernel DMAs
    ET_ORDER = [0, 4, 1, 5, 2, 6, 3, 7]
    # wq: [ct,p,et,e'] -> [p, eo, ct, e']
    wq_pre = np.ascontiguousarray(
        wqT[:, :1024].reshape(CT, P, ET, P).transpose(1, 2, 0, 3)[:, ET_ORDER]
    ).reshape(P, ET * CT * P)
    wqa_pre = np.ascontiguousarray(wq_pre[:, :1024])
    wqb_pre = np.ascontiguousarray(wq_pre[:, 1024:])
    wv_pre = np.ascontiguousarray(
        wqT[:, 1024:1536].reshape(CT, P, C).transpose(1, 0, 2)
    ).reshape(P, CT * C)
    wo_pre = np.ascontiguousarray(
        woT.reshape(CT, P, C).transpose(1, 0, 2)
    ).reshape(P, CT * C)
    # x: [ct,p,nt,s'] -> [p, nt, ct, s']
    x_pre = [
        np.ascontiguousarray(
            xb[n].reshape(CT, P, 2, 512).transpose(1, 2, 0, 3)
        ).reshape(P, 2 * CT * 512)
        for n in range(N)
    ]

    nc = get_program()
    in_maps = [
        {
            "xa": np.ascontiguousarray(x_pre[n][:, :2048]),
            "xb": np.ascontiguousarray(x_pre[n][:, 2048:]),
            "wqa": wqa_pre, "wqb": wqb_pre, "wv": wv_pre, "wo": wo_pre,
        }
        for n in range(N)
    ]
    res = run_bass_kernel_spmd(
        nc, in_maps, core_ids=list(range(N)), trace=_trace, tmpdir=_tmpdir
    )
    y = np.stack([res.results[n]["y"] for n in range(N)]).astype(np.float32)
    y = y.reshape(N, C, 32, 32)
    y = y + b_out[None, :, None, None]
    if _trace:
        return y, res
    return y


# revision 65
# speedup vs baseline: 1.0048x; 1.0048x over previous
"""MHA kernel for TRN2: x[8,512,32,32], 8 heads, S=1024, C=512.

Sharding: data-parallel over batch N=8 -> one batch item per NeuronCore.
Per-core layout (all transpose-free):
  qkT[e,s]  = w_qkvT[:, :1024].T @ x      (e on partitions; q tiles 0-3, k tiles 4-7)
  v[s,e]    = x.T @ w_qkvT[:, 1024:]      (s on partitions, natural layout)
  scoresT   = kT_h.T @ qT_h               (k_s on partitions; K=64 -> head pair packed
                                           at array rows 0-63 / 64-127, runs 2x via
                                           implicit 64x128 row tiling)
  P         = exp(scoresT * 1/8)          (ACT, batched 2048-wide from PSUM)
  oT_aug    = [v_h | 1].T @ P             (M=65; row 64 = softmax denominator r)
  oT        = oT_aug[:64] * (1/r)         (gpsimd partition_broadcast of 1/r)
  yT[o,s]   = w_outT.T @ oT               (+ b_out added host-side; == NCHW layout)

Perf structure (116.0us baseline -> ~111.9us):
  - inputs are host-preswizzled into contiguous-per-partition layouts and land
    in 4 consolidated SBUF tiles via 2KB-element DMA slices in needed-first
    order (wqa -> xa -> wqb/wv/wo on sync; xb on the scalar HWDGE queue):
    sub-2KB descriptors or >2KB elements both collapse DMA queue throughput
  - 12 warm-up matmuls on a zero tile bridge the DMA window so the PE HAM
    clock-gate is already 8/8 when the real stream starts
  - mid-kernel normalization: one staging copy [65,512] frees the pso bank in
    ~0.7us (the next step's PV accumulators are gated on it); all four staging
    copies are emitted before any finish chain so the in-order DVE releases
    banks back-to-back. The r row is re-staged to partition 0 of its own tile
    because custom-DVE ops and partition_broadcast ignore AP partition offsets
  - final drain: pair-3 PV runs the nt=1 indices first (bank-alternating; a
    per-bank sweep serializes on same-bank accumulation), norm chains read
    PSUM directly, st1 reuses the drained pso_t tiles in place; st0 ct0..2
    and the st1 groups fill the PE after the PV sweeps, and st0's ct=3 closes
    as two 64x128 row-tiled halves gated on one chain each. Final y copies
    split across ACT and DVE, all y DMAs on the idle sync queue
"""

import numpy as np
import ml_dtypes

import concourse.bacc as bacc
import concourse.mybir as mybir
import concourse.tile as tile
from concourse.bass_utils import run_bass_kernel_spmd

P = 128
S = 1024          # sequence = 32*32
C = 512           # channels
NH = 8            # heads
HD = 64           # head dim
CT = C // P       # 4 c-tiles
ET = 2 * C // P   # 8 e-tiles for q+k
MT = S // P       # 8 s-tiles
BF = mybir.dt.bfloat16
F32 = mybir.dt.float32

_cache = {}


def build_program(dbg=False):
    nc = bacc.Bacc("TRN2", target_bir_lowering=False, debug=False, num_devices=8)
    # inputs come pre-swizzled from the host so every DMA is a contiguous
    # >=2KB-per-partition read (small descriptors collapse queue throughput):
    #   x:  [p, nt, ct, 512]  = x[ct*128+p, nt*512+s]
    #   wq: [p, eo, ct, 128]  = w_qkvT[ct*128+p, ET_ORDER[eo]*128+e]
    #   wv: [p, ct, 512]      = w_qkvT[ct*128+p, 1024+e]
    #   wo: [p, ct, 512]      = w_outT[ct*128+p, o]
    xa_d = nc.dram_tensor("xa", [P, CT * 512], BF, kind="ExternalInput").ap()
    xb_d = nc.dram_tensor("xb", [P, CT * 512], BF, kind="ExternalInput").ap()
    wqa_d = nc.dram_tensor("wqa", [P, 2 * CT * P], BF, kind="ExternalInput").ap()
    wqb_d = nc.dram_tensor("wqb", [P, 6 * CT * P], BF, kind="ExternalInput").ap()
    wv_d = nc.dram_tensor("wv", [P, CT * C], BF, kind="ExternalInput").ap()
    wo_d = nc.dram_tensor("wo", [P, CT * C], BF, kind="ExternalInput").ap()
    y_d = nc.dram_tensor("y", [C, S], BF, kind="ExternalOutput").ap()

    with tile.TileContext(nc) as tc:
        with (
            tc.tile_pool(name="const", bufs=1) as cpool,
            tc.tile_pool(name="qk", bufs=1) as qkpool,
            tc.tile_pool(name="vp", bufs=1) as vpool,
            tc.tile_pool(name="pp", bufs=32) as ppool,
            tc.tile_pool(name="ot", bufs=1) as opool,
            tc.tile_pool(name="yp", bufs=1) as ypool,
            tc.tile_pool(name="misc", bufs=4) as mpool,
            tc.tile_pool(name="psq", bufs=2, space="PSUM") as psq_pool,
            tc.tile_pool(name="pso", bufs=4, space="PSUM") as pso_pool,
        ):
            # ---- warm-up: zero tile + dummy matmuls keep the PE busy while
            # input DMAs stream, so HAM is un-throttled for the real work ----
            wz = cpool.tile([P, 512], BF, name="wz", tag="wz")
            nc.gpsimd.memset(wz[:], 0.0)
            wu = psq_pool.tile([P, 1024], F32, name="wu", tag="psq")
            for _ in range(12):
                nc.tensor.matmul(wu[:, 0:512], wz[:, 0:128], wz[:], start=True, stop=True)

            # ---- consolidated input tiles, same layout as the prepped DRAM ----
            w4 = cpool.tile([P, ET * CT * P], BF, name="w4", tag="w4")
            x4 = cpool.tile([P, 2 * CT * 512], BF, name="x4", tag="x4")
            wv4 = cpool.tile([P, CT * C], BF, name="wv4", tag="wv4")
            wo4 = cpool.tile([P, CT * C], BF, name="wo4", tag="wo4")

            # One queue, strict priority order; every dma_start moves a
            # 1024-column (2KB-per-partition) slice - that elem size keeps the
            # queue streaming back-to-back, and a second parallel queue only
            # splits HBM bandwidth without improving the critical chain.
            nc.sync.dma_start(w4[:, 0:1024], wqa_d[:])             # et 0 and 4
            nc.sync.dma_start(x4[:, 0:1024], xa_d[:, 0:1024])      # s 0:512
            nc.sync.dma_start(x4[:, 1024:2048], xa_d[:, 1024:2048])
            for i in range(2):
                sl = slice(i * 1024, (i + 1) * 1024)
                nc.scalar.dma_start(x4[:, 2048 + i * 1024:3072 + i * 1024],
                                    xb_d[:, sl])                   # s 512:1024
            for i in range(3):                                     # et 1,5|2,6|3,7
                sl = slice(i * 1024, (i + 1) * 1024)
                nc.sync.dma_start(w4[:, 1024 + i * 1024:2048 + i * 1024], wqb_d[:, sl])
            for i in range(2):
                sl = slice(i * 1024, (i + 1) * 1024)
                nc.sync.dma_start(wv4[:, sl], wv_d[:, sl])
            for i in range(2):
                sl = slice(i * 1024, (i + 1) * 1024)
                nc.sync.dma_start(wo4[:, sl], wo_d[:, sl])

            ET_ORDER = (0, 4, 1, 5, 2, 6, 3, 7)
            ET_OFF = {et: i for i, et in enumerate(ET_ORDER)}

            def w_slice(et, ct):
                o = (ET_OFF[et] * CT + ct) * P
                return w4[:, o:o + P]

            def x_nt(ct, nt):
                o = (nt * CT + ct) * 512
                return x4[:, o:o + 512]

            def x_mt(ct, mt):
                nt, r = divmod(mt, 4)
                o = (nt * CT + ct) * 512 + r * P
                return x4[:, o:o + P]

            def wv_sb(ct):
                return wv4[:, ct * C:(ct + 1) * C]

            def wo_sb(ct):
                return wo4[:, ct * C:(ct + 1) * C]

            # ---- qkT projection: [e=1024 rows, s=1024] ----
            qk_sb = []
            for et in range(ET):
                t = qkpool.tile([P, S], BF, name=f"qk{et}", tag=f"qk{et}")
                qk_sb.append(t)
            v_sb = [None] * MT

            def emit_qkv_group(et, nt):
                ps = pso_pool.tile([P, 512], F32, name="qp", tag="pso")
                for ct in range(CT):
                    nc.tensor.matmul(
                        ps[:],
                        w_slice(et, ct),
                        x_nt(ct, nt),
                        start=(ct == 0), stop=(ct == CT - 1),
                    )
                nc.vector.tensor_copy(qk_sb[et][:, nt * 512:(nt + 1) * 512], ps[:])

            def emit_v_group(mt):
                ps = pso_pool.tile([P, 512], F32, name="vp", tag="pso")
                for ct in range(CT):
                    nc.tensor.matmul(
                        ps[:],
                        x_mt(ct, mt),
                        wv_sb(ct),
                        start=(ct == 0), stop=(ct == CT - 1),
                    )
                vt = vpool.tile([P, NH * (HD + 1)], BF, name=f"v{mt}", tag=f"v{mt}")
                vv = vt[:].rearrange("p (h e) -> p h e", e=HD + 1)
                nc.gpsimd.memset(vv[:, :, HD:HD + 1], 1.0)
                nc.vector.tensor_copy(
                    vv[:, :, 0:HD], ps[:].rearrange("p (h e) -> p h e", e=HD))
                v_sb[mt] = vt

            # block A: only the nt0 tiles gate the first QK/exp; the nt1
            # groups are emitted right after the first ACT (see step loop)
            for et, nt in ((0, 0), (4, 0)):
                emit_qkv_group(et, nt)
            pending = [("qkv", et, nt) for et in (1, 5, 2, 6, 3, 7) for nt in (0, 1)]
            pending += [("v", mt, None) for mt in range(MT)]
            pend_i = 0

            # ---- attention, software-pipelined: QK/exp(pair p) || PV(pair p-1);
            #      step 0 also drains the remaining qkv/v projection groups ----
            oT_sb = [opool.tile([P, S], BF, name=f"o{ct}", tag=f"o{ct}") for ct in range(CT)]
            p_tiles = {}
            DRAIN_ORDER = ((0, 0), (1, 0), (0, 1), (1, 1))
            LAST_ORDER = ((0, 1), (1, 1), (0, 0), (1, 0))

            def norm_stage(pso_t, idx):
                # phase A: one fast DVE copy stages the whole PSUM tile to
                # SBUF so the pso bank frees in ~0.7us (the next step's PV
                # accumulations are gated on it)
                stage = mpool.tile([HD + 1, 512], F32, name="stage", tag="stage")
                nc.vector.tensor_copy(stage[:], pso_t[idx][0:HD + 1, :])
                return stage

            def norm_finish_pre(stage):
                # phase B1 off SBUF: the r row is re-staged to partition 0 of
                # its own tile (custom-DVE ops and partition_broadcast read
                # partition 0 of the underlying tile regardless of slicing)
                rrow = mpool.tile([1, 512], F32, name="rrow", tag="rrow")
                nc.vector.tensor_copy(rrow[0:1, :], stage[HD:HD + 1, :])
                rinv = mpool.tile([1, 512], F32, name="rinv", tag="rinv")
                nc.vector.reciprocal_approx_fast(rinv[0:1, :], rrow[0:1, :])
                bc = mpool.tile([HD, 512], F32, name="bc", tag="bc")
                nc.gpsimd.partition_broadcast(bc[:], rinv[0:1, :], channels=HD)
                return bc

            def norm_finish_mul(step, idx_order_idx, stage, bc, order):
                pp = step - 1
                hh, nt = order[idx_order_idx]
                h = 2 * pp + hh
                ct, half = h // 2, h % 2
                nc.vector.tensor_mul(
                    oT_sb[ct][half * HD:(half + 1) * HD, nt * 512:(nt + 1) * 512],
                    stage[0:HD, :], bc[:],
                )

            def norm_finish(step, idx_order_idx, stage, order):
                bc = norm_finish_pre(stage)
                norm_finish_mul(step, idx_order_idx, stage, bc, order)

            def emit_norm(step, idx_order_idx, pso_t, order):
                stage = norm_stage(pso_t, idx_order_idx)
                norm_finish(step, idx_order_idx, stage, order)

            def norm_direct_pre(pso_t, idx):
                # drain-tail variant: no staging (nothing else wants the pso
                # bank sooner than the chain finishes) - rrow/recip only
                rrow = mpool.tile([1, 512], F32, name="rrow", tag="rrow")
                nc.vector.tensor_copy(rrow[0:1, :], pso_t[idx][HD:HD + 1, :])
                rinv = mpool.tile([1, 512], F32, name="rinv", tag="rinv")
                nc.vector.reciprocal_approx_fast(rinv[0:1, :], rrow[0:1, :])
                bc = mpool.tile([HD, 512], F32, name="bc", tag="bc")
                nc.gpsimd.partition_broadcast(bc[:], rinv[0:1, :], channels=HD)
                return bc

            def norm_direct_mul(step, idx_order_idx, pso_t, bc, order):
                pp = step - 1
                hh, nt = order[idx_order_idx]
                h = 2 * pp + hh
                ct, half = h // 2, h % 2
                nc.vector.tensor_mul(
                    oT_sb[ct][half * HD:(half + 1) * HD, nt * 512:(nt + 1) * 512],
                    pso_t[idx_order_idx][0:HD, :], bc[:],
                )

            y_sb = [ypool.tile([P, S], BF, name=f"y{ot}", tag=f"y{ot}") for ot in range(CT)]

            for step in range(NH // 2):
                pso_t = None
                if step >= 1:
                    pso_t = [pso_pool.tile([P, 512], F32, name=f"pso{i}", tag="pso")
                             for i in range(4)]
                for mt in range(MT):
                    for nt in range(2):
                        psq = psq_pool.tile([P, 1024], F32, name="psq", tag="psq")
                        for hh in range(2):
                            nc.tensor.matmul(
                                psq[:, hh * 512:(hh + 1) * 512],
                                qk_sb[4 + step][hh * HD:(hh + 1) * HD, mt * P:(mt + 1) * P],
                                qk_sb[step][hh * HD:(hh + 1) * HD, nt * 512:(nt + 1) * 512],
                                start=True, stop=True,
                            )
                        pt = ppool.tile([P, 1024], BF, name="ptile", tag="ptile")
                        nc.scalar.activation(
                            pt[:], psq[:], mybir.ActivationFunctionType.Exp,
                            scale=float(1.0 / np.sqrt(HD)),
                        )
                        p_tiles[(step, mt, nt)] = pt
                        if step == 0 and mt == 0 and nt == 0:
                            emit_qkv_group(0, 1)
                            emit_qkv_group(4, 1)
                        if step == 0:
                            slot = mt * 2 + nt
                            want = 20 * (slot + 1) // 16
                            while pend_i < min(want, 20):
                                kind, i1, i2 = pending[pend_i]
                                if kind == "qkv":
                                    emit_qkv_group(i1, i2)
                                else:
                                    emit_v_group(i1)
                                pend_i += 1
                    if step >= 1:
                        pp = step - 1
                        for idx, (hh, nt) in enumerate(DRAIN_ORDER):
                            h = 2 * pp + hh
                            nc.tensor.matmul(
                                pso_t[idx][0:HD + 1, :],
                                v_sb[mt][:, h * (HD + 1):(h + 1) * (HD + 1)],
                                p_tiles[(pp, mt, nt)][:, hh * 512:(hh + 1) * 512],
                                start=(mt == 0), stop=(mt == MT - 1),
                            )
                if step >= 1:
                    # all four staging copies first: they are what frees the
                    # pso ring for the next step's PV; the serial finish
                    # chains would otherwise delay stages 2/3 on the DVE FIFO
                    stages = [norm_stage(pso_t, i) for i in range(4)]
                    for i in range(4):
                        norm_finish(step, i, stages[i], DRAIN_ORDER)

            # ---- pair-3 drain + output projection ----
            # PV runs in two passes over mt: the nt=1 drain indices first, so
            # their normalization chains and the st1 projection groups hide
            # under the nt=0 PV pass; st0 (columns 0:512) accumulates ct 0..2
            # on the freed psq ring meanwhile.
            pp = NH // 2 - 1
            pso_t = [pso_pool.tile([P, 512], F32, name=f"psoF{i}", tag="pso")
                     for i in range(4)]

            def pv_last(idx):
                hh, nt = LAST_ORDER[idx]
                h = 2 * pp + hh
                for mt in range(MT):
                    nc.tensor.matmul(
                        pso_t[idx][0:HD + 1, :],
                        v_sb[mt][:, h * (HD + 1):(h + 1) * (HD + 1)],
                        p_tiles[(pp, mt, nt)][:, hh * 512:(hh + 1) * 512],
                        start=(mt == 0), stop=(mt == MT - 1),
                    )

            psA = psq_pool.tile([P, 1024], F32, name="prA", tag="psq")
            psB = psq_pool.tile([P, 1024], F32, name="prB", tag="psq")

            # filler matmuls occupy the PE while step-3's staging copies
            # release the pso ring for pass A - without them the ~1.7us idle
            # window re-throttles the HAM clock-gate to 4/8 for the whole
            # drain. st0's first accumulation (start=True) overwrites psA.
            for _ in range(5):
                nc.tensor.matmul(psA[:, 0:512], wz[:, 0:128], wz[:],
                                 start=True, stop=True)

            def st0_ps(g):
                t = psA if g < 2 else psB
                return t[:, (g % 2) * 512:(g % 2 + 1) * 512]

            def emit_st1(g, ps):
                # accumulates in the drained pso_t[g] tile, in place
                for ct in range(CT):
                    nc.tensor.matmul(
                        ps[:],
                        wo_sb(ct)[:, g * P:(g + 1) * P],
                        oT_sb[ct][:, 512:1024],
                        start=(ct == 0), stop=(ct == CT - 1),
                    )
                dst = y_sb[g][:, 512:1024]
                nc.scalar.copy(dst, ps[:])
                nc.sync.dma_start(y_d[g * P:(g + 1) * P, 512:1024], dst)

            # pass A: nt=1 indices together (they share P tiles per mt)
            for mt in range(MT):
                nc.tensor.matmul(
                    pso_t[0][0:HD + 1, :],
                    v_sb[mt][:, (2 * pp) * (HD + 1):(2 * pp + 1) * (HD + 1)],
                    p_tiles[(pp, mt, 1)][:, 0:512],
                    start=(mt == 0), stop=(mt == MT - 1),
                )
                nc.tensor.matmul(
                    pso_t[1][0:HD + 1, :],
                    v_sb[mt][:, (2 * pp + 1) * (HD + 1):(2 * pp + 2) * (HD + 1)],
                    p_tiles[(pp, mt, 1)][:, 512:1024],
                    start=(mt == 0), stop=(mt == MT - 1),
                )
            # direct (no-staging) norm chains, rrow/recip pairs interleaved so
            # the DVE never waits on a gpsimd broadcast
            bc0 = norm_direct_pre(pso_t, 0)
            bc1 = norm_direct_pre(pso_t, 1)
            norm_direct_mul(NH // 2, 0, pso_t, bc0, LAST_ORDER)
            norm_direct_mul(NH // 2, 1, pso_t, bc1, LAST_ORDER)

            # pass B PV sweeps run back-to-back; every normalization chain
            # and the st0/st1 filler matmuls hide underneath them
            pv_last(2)
            bc2 = norm_direct_pre(pso_t, 2)
            norm_direct_mul(NH // 2, 2, pso_t, bc2, LAST_ORDER)
            pv_last(3)

            # st0 accumulation over ct=0..2 has no dependency on the last
            # pair; by now the step-3 norms it reads are long done
            for ct in range(CT - 1):
                for g in range(CT):
                    nc.tensor.matmul(
                        st0_ps(g),
                        wo_sb(ct)[:, g * P:(g + 1) * P],
                        oT_sb[ct][:, 0:512],
                        start=(ct == 0), stop=False,
                    )
            emit_st1(0, pso_t[0])
            emit_st1(1, pso_t[1])
            bc3 = norm_direct_pre(pso_t, 3)
            norm_direct_mul(NH // 2, 3, pso_t, bc3, LAST_ORDER)
            emit_st1(2, pso_t[2])
            emit_st1(3, pso_t[3])
            # st0's ct=3 contraction splits into row halves (64x128 row-tiled
            # matmuls) so the first half runs as soon as the idx2 chain's
            # multiply lands, instead of waiting for idx3's as well
            ct = CT - 1
            for g in range(CT):
                nc.tensor.matmul(
                    st0_ps(g),
                    wo_sb(ct)[0:HD, g * P:(g + 1) * P],
                    oT_sb[ct][0:HD, 0:512],
                    start=False, stop=False,
                )
            for g in range(CT):
                nc.tensor.matmul(
                    st0_ps(g),
                    wo_sb(ct)[HD:P, g * P:(g + 1) * P],
                    oT_sb[ct][HD:P, 0:512],
                    start=False, stop=True,
                )
            # split the final evacuation across ACT and DVE so the last four
            # copies run pairwise-parallel instead of serializing on one engine
            for g in range(CT):
                dst = y_sb[g][:, 0:512]
                if g < 2:
                    nc.scalar.copy(dst, st0_ps(g))
                else:
                    nc.vector.tensor_copy(dst, st0_ps(g))
                nc.sync.dma_start(y_d[g * P:(g + 1) * P, 0:512], dst)

    nc.compile()
    return nc


def get_program():
    if "nc" not in _cache:
        _cache["nc"] = build_program()
    return _cache["nc"]


def kernel(x, w_qkv, w_out, b_out, _trace=False, _tmpdir=None):
    x = np.asarray(x, dtype=np.float32)
    w_qkv = np.asarray(w_qkv, dtype=np.float32)
    w_out = np.asarray(w_out, dtype=np.float32)
    b_out = np.asarray(b_out, dtype=np.float32)
    N = x.shape[0]

    xb = x.reshape(N, C, S).astype(ml_dtypes.bfloat16)
    wqT = np.ascontiguousarray(w_qkv.T).astype(ml_dtypes.bfloat16)
    woT = np.ascontiguousarray(w_out.T).astype(ml_dtypes.bfloat16)

    # pre-swizzle into the contiguous-per-partition layouts the kernel DMAs
    ET_ORDER = [0, 4, 1, 5, 2, 6, 3, 7]
    # wq: [ct,p,et,e'] -> [p, eo, ct, e']
    wq_pre = np.ascontiguousarray(
        wqT[:, :1024].reshape(CT, P, ET, P).transpose(1, 2, 0, 3)[:, ET_ORDER]
    ).reshape(P, ET * CT * P)
    wqa_pre = np.ascontiguousarray(wq_pre[:, :1024])
    wqb_pre = np.ascontiguousarray(wq_pre[:, 1024:])
    wv_pre = np.ascontiguousarray(
        wqT[:, 1024:1536].reshape(CT, P, C).transpose(1, 0, 2)
    ).reshape(P, CT * C)
    wo_pre = np.ascontiguousarray(
        woT.reshape(CT, P, C).transpose(1, 0, 2)
    ).reshape(P, CT * C)
    # x: [ct,p,nt,s'] -> [p, nt, ct, s']
    x_pre = [
        np.ascontiguousarray(
            xb[n].reshape(CT, P, 2, 512).transpose(1, 2, 0, 3)
        ).reshape(P, 2 * CT * 512)
        for n in range(N)
    ]

    nc = get_program()
    in_maps = [
        {
            "xa": np.ascontiguousarray(x_pre[n][:, :2048]),
            "xb": np.ascontiguousarray(x_pre[n][:, 2048:]),
            "wqa": wqa_pre, "wqb": wqb_pre, "wv": wv_pre, "wo": wo_pre,
        }
        for n in range(N)
    ]
    res = run_bass_kernel_spmd(
        nc, in_maps, core_ids=list(range(N)), trace=_trace, tmpdir=_tmpdir
    )
    y = np.stack([res.results[n]["y"] for n in range(N)]).astype(np.float32)
    y = y.reshape(N, C, 32, 32)
    y = y + b_out[None, :, None, None]
    if _trace:
        return y, res
    return y
